# revision 17
# baseline (speedup 1.0000x reference)
"""DiscreteKDE kernel for 8 Trainium2 NeuronCores.

Full computation:
    Q = 64; H_I = inv(H_bandwidth)
    Z  = (idx[:,None]-idx[None,:]) @ H_I
    KW = (1/sqrt(2pi)) * exp(-0.5 * Z*Z)
    col_sums = concat([X_probs.sum(0), Y_probs.sum(0)])     # (64,)  <- 256MB read
    T  = dot(KW.sum(0), col_sums)
    out = T * jnp.ones((256,256,256))                        # 67MB write

Hard-won structure notes (per-core):
  - HBM read throughput is ~335-358 GB/s per core NO MATTER HOW MANY
    HWDGE rings issue (measured: one ring 335; two rings on disjoint
    tiles 340; two rings split within a tile 220 -- the same-partition
    descriptor pairs collide). So the 32MB stream rides ONE ring (sync)
    as 15 x [128, 4096] tiles (16KB descs, fan-out 16), strict FIFO so
    DVE's in-order consume never stalls on a lagging second ring.
  - The 0.5MB tails ([128,1024] + [72,64]) ride the otherwise-idle
    scalar ring EARLY, so the sync FIFO ends with tile 14 and nothing
    small trails the stream.
  - Small inputs (64x64) go on the gpsimd SWDGE ring: 256B descriptors
    would eat ~2.9us of HWDGE issue each and delay the stream start.
  - DVE fp32 tensor_tensor is ~1.05ns/elem/lane + ~154ns fixed, so
    tiles 0-13 are consumed as [128,2048] wide adds into acc_w
    (2.3us/MB = 455GB/s > stream rate; v1's [128,512] adds made DVE a
    co-bottleneck). Tiles 13+14 are col-split (2x8KB / 4x4KB sub-DMAs)
    and consumed narrow into acc2 [128,512] so the narrow adds chase
    the stream per-slice and the LAST PSUM fold is a single matmul:
    the wide fold (4 matmuls, ones_k^T @ acc_w chunks accumulating
    into ps_pe [1,512]) hides behind tile 12, the acc2 fold closes
    the accumulation.
  - The dot d = <rep8(KW.sum(0)), ps_pe> runs on DVE reading PSUM
    directly (mul + X-reduce); the 128-partition broadcast of d comes
    back via PE and lands in SBUF via a DVE copy (gpsimd can't touch
    PSUM but must source the remote writes from SBUF).
  - Newton-Schulz inverse of H on PE+ACT (iterating on the negated
    inverse R' = R A R + 2R), alpha chain on gpsimd/ACT (1/x as
    exp(-ln x)); all done by ~35us, far off the critical path.
  - cross-core sum of the per-core dot d_r: HAND-ROLLED flat all-gather
    via 7 remote_dma_broadcast preps (dest j in slot j so cross-die
    dests ride D2D lanes), one trigger_dma gated on d (add_dep_helper;
    remote preps are "user-synced" so the framework defers NOTHING),
    then a DVE reduce of the [128, 8] gather tile. The recv wait
    (rsem >= 14; each inbound write bumps +2) is spliced into the
    instruction list POST-scheduling: the single-core tile scheduler
    deadlocks on a visible cross-core wait and hoists a depless
    placeholder to the engine's idle front (both tried, both failed).
    Replaces the runtime AllGather whose small-payload latency floor is
    ~20us (trigger delay + mesh walk) with ~2-4us of peer SBUF writes.
  - fill: [128, 4096] tile * T (16KB descs), one 4MB broadcast-AP DMA
    per ring; HBM WRITES do reach ~420GB/s across two rings.
  - Launch skew between the 8 PJRT dispatches is ~5-6us/core (varies);
    every core waits for the straggler's d, so the first-launched core
    pays the full spread. Unfixable from inside one SPMD program;
    everything else is tuned so the straggler's own timeline is short.
"""

import os
import sys

import numpy as np

for _p in ("/opt/trn_rl_repo", "/root/.axon_site/_ro/trn_rl_repo"):
    if os.path.isdir(_p) and _p not in sys.path:
        sys.path.insert(0, _p)

import concourse.bacc as bacc
import concourse.bass as bass
import concourse.mybir as mybir
from concourse.bass_utils import run_bass_kernel_spmd
from concourse.tile import TileContext
from concourse.tile_rust import add_dep_helper

# ---- problem constants (hardcoded per spec) ----
N_TOTAL = 1_000_000
FDIM = 61
HDIM = 3
Q = 64                      # FDIM + HDIM
KGRID = 256
HOUT = 3
NCORES = 8
ROWS_PER_CORE = N_TOTAL // NCORES          # 125000

# ---- tiling ----
P = 128
G = 64                      # rows/partition/tile -> 16KB descriptors
NT = 15                     # full tiles: 15 * 128 * 64 = 122880 rows
TW = G * Q                  # 4096 f32 = 16KB per partition per tile
WCH = 2048                  # wide DVE chunk
CH = 512                    # narrow chunk = residual-group space
NCHUNK = TW // CH           # 8
MAIN_ROWS = NT * P * G      # 122880
TAILA_G = 16                # [128, 1024]: 2048 rows, 4KB descriptors
TAILA_ROWS = P * TAILA_G    # 2048
TAILB_ROWS = ROWS_PER_CORE - MAIN_ROWS - TAILA_ROWS   # 72
STREAM_BUFS = 7
LAST_TILE = NT - 1          # consumed narrow into acc2

OUT_TOTAL = KGRID ** HOUT                  # 16_777_216
OUT_PER_CORE = OUT_TOTAL // NCORES         # 2_097_152
# 2048 (8KB descs) not 4096: same chip-capped fill bandwidth (issue rate
# is not the limiter), but the post-wait T-multiply halves to ~1.2us
FILL_W = 2048
N_FILL = OUT_PER_CORE // (P * FILL_W)      # 8

NEWTON_ITERS = 11
INV_SQRT_2PI = 0.3989422804014327
LN_C = float(np.log(INV_SQRT_2PI))

F32 = mybir.dt.float32
AX = mybir.AxisListType
ALU = mybir.AluOpType
ACT_FN = mybir.ActivationFunctionType


def build_nc(use_remote_exchange=True):
    nc = bacc.Bacc("TRN2", target_bir_lowering=False, debug=False,
                   num_devices=NCORES)

    c_in = nc.dram_tensor("c", [ROWS_PER_CORE, Q], F32, kind="ExternalInput")
    h_in = nc.dram_tensor("h", [Q, Q], F32, kind="ExternalInput")
    out = nc.dram_tensor("o", [OUT_PER_CORE], F32, kind="ExternalOutput")

    idx = np.arange(Q, dtype=np.float64)
    d_const = nc.inline_tensor(
        (idx[:, None] - idx[None, :]).astype(np.float32), "dmat")
    i2_const = nc.inline_tensor(
        (2.0 * np.eye(Q)).astype(np.float32), "i2mat")
    n2_const = nc.inline_tensor(
        (-2.0 * np.eye(Q)).astype(np.float32), "n2mat")

    if not use_remote_exchange:
        cc_in = nc.dram_tensor("cc_in", [1], F32)
        cc_out = nc.dram_tensor("cc_out", [NCORES], F32, addr_space="Shared")

    with TileContext(nc) as tc:
        with (
            tc.tile_pool(name="const", bufs=1) as cpool,
            tc.tile_pool(name="stream", bufs=STREAM_BUFS) as spool,
            tc.tile_pool(name="small", bufs=2) as mpool,
            tc.tile_pool(name="accp", bufs=1, space=bass.MemorySpace.PSUM) as ppool,
            tc.tile_pool(name="psmall", bufs=2, space=bass.MemorySpace.PSUM) as pspool,
        ):
            # ---------- stream DMAs (sync ring, strict FIFO) ----------
            cv = c_in.ap()[:MAIN_ROWS, :].rearrange(
                "(t p g) q -> t p (g q)", t=NT, p=P, g=G)
            taila_v = c_in.ap()[MAIN_ROWS:MAIN_ROWS + TAILA_ROWS, :].rearrange(
                "(p g) q -> p (g q)", p=P, g=TAILA_G)
            tailb_v = c_in.ap()[MAIN_ROWS + TAILA_ROWS:, :]

            tiles = []
            taila_t = cpool.tile([P, TAILA_G * Q], F32)
            tailb_t = cpool.tile([TAILB_ROWS, Q], F32)
            for t in range(NT):
                st = spool.tile([P, TW], F32, tag="stream")
                if t == LAST_TILE:
                    # column-split the last tile into 4 sub-DMAs (4KB
                    # descs; issue-ahead hides the desc-rate cost) so the
                    # narrow DVE adds chase the stream instead of waiting
                    # for the whole 2MB to land
                    for b in range(4):
                        nc.sync.dma_start(st[:, b * 1024:(b + 1) * 1024],
                                          cv[t][:, b * 1024:(b + 1) * 1024])
                elif t == LAST_TILE - 1:
                    # t13 col-split in halves and consumed narrow too, so
                    # the 4-matmul acc_w fold hides behind t12 instead of
                    # gating the tail
                    for b in range(2):
                        nc.sync.dma_start(st[:, b * 2048:(b + 1) * 2048],
                                          cv[t][:, b * 2048:(b + 1) * 2048])
                else:
                    nc.sync.dma_start(st[:], cv[t])
                tiles.append(st)
                if t == 0:
                    # tails on the idle scalar ring, arriving early;
                    # dedicated buffers, consumed on arrival
                    nc.scalar.dma_start(taila_t[:], taila_v)
                    nc.scalar.dma_start(tailb_t[:], tailb_v)

            # ---------- constants (gpsimd: DVE streams, ACT does Newton) --
            ones_k = cpool.tile([P, 1], F32)        # lhsT partition-reduce
            nc.gpsimd.memset(ones_k[:], 1.0)
            ones_row = cpool.tile([1, P], F32)      # lhsT bcast scalar->128
            nc.gpsimd.memset(ones_row[:], 1.0)
            ones_q = cpool.tile([Q, 1], F32)        # lhsT 64-part reduce
            nc.gpsimd.memset(ones_q[:], 1.0)
            ones_rq = cpool.tile([1, Q], F32)       # lhsT bcast scalar->64
            nc.gpsimd.memset(ones_rq[:], 1.0)
            lnc = cpool.tile([Q, 1], F32)           # exp bias ln(1/sqrt 2pi)
            nc.gpsimd.memset(lnc[:], LN_C)
            ones_fill = cpool.tile([P, FILL_W], F32)
            nc.gpsimd.memset(ones_fill[:], 1.0)

            # ---------- small inputs (gpsimd SWDGE: tiny descriptors) -----
            a_t = cpool.tile([Q, Q], F32)
            nc.gpsimd.dma_start(a_t[:], h_in.ap())
            d_t = cpool.tile([Q, Q], F32)
            nc.gpsimd.dma_start(d_t[:], d_const.ap())
            i2_t = cpool.tile([Q, Q], F32)
            nc.gpsimd.dma_start(i2_t[:], i2_const.ap())
            n2_t = cpool.tile([Q, Q], F32)
            nc.gpsimd.dma_start(n2_t[:], n2_const.ap())

            # ---------- exchange state ----------
            if use_remote_exchange:
                rsem = nc.alloc_semaphore("xg_rsem")
                lsem = nc.alloc_semaphore("xg_lsem")
                ag = cpool.tile([P, NCORES], F32)    # gathered d_r
                db = cpool.tile([P, 1], F32)         # my d on 128 parts

            # ---------- alpha chain (gpsimd + ACT + PE) ----------
            # tmp_qq = A * 2I is diagonal => 2*trace = full-tensor sum,
            # exactly gpsimd's XYZWC (partition-inclusive) reduce.
            tmp_qq = mpool.tile([Q, Q], F32, tag="qq")
            nc.gpsimd.tensor_mul(tmp_qq[:], a_t[:], i2_t[:])
            tr_s = mpool.tile([1, 1], F32, tag="q1")
            nc.gpsimd.tensor_reduce(tr_s[:], tmp_qq[:], axis=AX.XYZWC,
                                    op=ALU.add)
            ln_t = mpool.tile([1, 1], F32, tag="s11ln")
            nc.scalar.activation(ln_t[:], tr_s[:], ACT_FN.Ln)
            tr2 = mpool.tile([1, 1], F32, tag="s11")
            nc.scalar.activation(tr2[:], ln_t[:], ACT_FN.Exp, scale=-1.0)
            ps_a = pspool.tile([Q, 1], F32, tag="ps_small")
            nc.tensor.matmul(ps_a[:], ones_rq[:], tr2[:])     # bcast->(64,1)
            al64 = mpool.tile([Q, 1], F32, tag="q1b")
            nc.scalar.activation(al64[:], ps_a[:], ACT_FN.Copy)

            # ---------- Newton-Schulz on PE + ACT ----------
            s_cur = mpool.tile([Q, Q], F32, tag="newton")
            nc.gpsimd.tensor_scalar_mul(s_cur[:], n2_t[:], al64[:])
            for it in range(NEWTON_ITERS):
                ps_y = pspool.tile([Q, Q], F32, tag="ps_qq")
                nc.tensor.matmul(ps_y[:], a_t[:], s_cur[:])       # A @ R
                y_sb = mpool.tile([Q, Q], F32, tag="newton_y")
                nc.scalar.activation(y_sb[:], ps_y[:], ACT_FN.Copy)
                ps_x = pspool.tile([Q, Q], F32, tag="ps_qq")
                nc.tensor.matmul(ps_x[:], s_cur[:], y_sb[:],
                                 start=True, stop=False)          # R A R
                nc.tensor.matmul(ps_x[:], s_cur[:], i2_t[:],
                                 start=False, stop=True)          # + 2 R
                s_nxt = mpool.tile([Q, Q], F32, tag="newton")
                nc.scalar.activation(s_nxt[:], ps_x[:], ACT_FN.Copy)
                s_cur = s_nxt
            # Z = D.T @ (-H^-1) up to sign; KW = exp(-Z^2/2 + ln c)
            ps_z = pspool.tile([Q, Q], F32, tag="ps_qq")
            nc.tensor.matmul(ps_z[:], d_t[:], s_cur[:])
            z2 = mpool.tile([Q, Q], F32, tag="qq2")
            nc.scalar.square(z2[:], ps_z[:])
            kw = mpool.tile([Q, Q], F32, tag="qq3")
            nc.scalar.activation(kw[:], z2[:], ACT_FN.Exp,
                                 bias=lnc[:], scale=-0.5)
            ps_s = pspool.tile([1, Q], F32, tag="ps_small")
            nc.tensor.matmul(ps_s[:], ones_q[:], kw[:])           # KW.sum(0)
            # replicate KW.sum(0) 8x into the 512-wide residual space
            s_rep = mpool.tile([1, NCHUNK * Q], F32, tag="vec2")
            s_rep_v = s_rep[:].rearrange("p (g q) -> p g q", g=NCHUNK, q=Q)
            ps_s_b = ps_s[:].unsqueeze(1).broadcast_to([1, NCHUNK, Q])
            nc.scalar.activation(s_rep_v, ps_s_b, ACT_FN.Copy)

            # ---------- DVE consume: wide adds, narrow tail ----------
            acc_w = cpool.tile([P, WCH], F32)       # wide acc (t0..t13)
            acc2 = cpool.tile([P, CH], F32)         # narrow acc (tails+t14)
            ps_pe = ppool.tile([1, CH], F32)        # final accumulation

            last_dve = [None]
            dve_w = [0]
            acc2_n = [0]

            def dve_wide(tile):
                for b in range(TW // WCH):
                    sl = tile[:, b * WCH:(b + 1) * WCH]
                    if dve_w[0] == 0:
                        last_dve[0] = nc.vector.tensor_copy(acc_w[:], sl)
                    else:
                        last_dve[0] = nc.vector.tensor_add(
                            acc_w[:], acc_w[:], sl)
                    dve_w[0] += 1

            def dve_narrow(tile, ncols):
                for b in range(ncols // CH):
                    sl = tile[:, b * CH:(b + 1) * CH]
                    if acc2_n[0] == 0:
                        last_dve[0] = nc.vector.tensor_copy(acc2[:], sl)
                    else:
                        last_dve[0] = nc.vector.tensor_add(
                            acc2[:], acc2[:], sl)
                    acc2_n[0] += 1

            for t in range(NT - 2):
                dve_wide(tiles[t])
                if t == 1:
                    # tails arrived early on the scalar ring
                    dve_narrow(taila_t, TAILA_G * Q)
                    # tailB: 72 rows of 64 cols, q-aligned on acc2 group 0
                    last_dve[0] = nc.vector.tensor_add(
                        acc2[:TAILB_ROWS, :Q], acc2[:TAILB_ROWS, :Q],
                        tailb_t[:])
                    acc2_n[0] += 1
            # wide-acc fold: 4 matmuls, hidden behind the t13/t14 stream
            for m in range(WCH // CH):
                nc.tensor.matmul(ps_pe[:], ones_k[:],
                                 acc_w[:, m * CH:(m + 1) * CH],
                                 start=(m == 0), stop=False)
            # last two tiles narrow so the final fold is one matmul
            dve_narrow(tiles[LAST_TILE - 1], TW)
            dve_narrow(tiles[LAST_TILE], TW)
            nc.tensor.matmul(ps_pe[:], ones_k[:], acc2[:],
                             start=False, stop=True)

            # ---------- local dot d = <KW.sum(0), col sums> (DVE) ---------
            # (tensor_tensor_reduce would fuse these two ops but dies on
            # hardware -- JaxRuntimeError INTERNAL -- keep mul + reduce)
            dprod = mpool.tile([1, NCHUNK * Q], F32, tag="vec2b")
            nc.vector.tensor_mul(dprod[:], s_rep[:], ps_pe[:])
            d_loc = mpool.tile([1, 1], F32, tag="s11d")
            dred = nc.vector.tensor_reduce(d_loc[:], dprod[:], axis=AX.X,
                                           op=ALU.add)
            last_dve[0] = dred

            if use_remote_exchange:
                # broadcast d to 128 partitions, then fire the peer writes
                ps_b = pspool.tile([P, 1], F32, tag="ps_small")
                nc.tensor.matmul(ps_b[:], ones_row[:], d_loc[:])
                db_cp = nc.vector.tensor_copy(db[:], ps_b[:])
                nc.gpsimd.tensor_copy(ag[:, 0:1], db[:])          # self slot
                # preps are descriptor-gen only; the SDMA reads db when the
                # trigger fires, so only the trigger needs the data dep.
                for j in range(1, NCORES):
                    rd = [None] * NCORES
                    rd[j] = (0, j)
                    nc.gpsimd.remote_dma_broadcast(
                        ag[:, j:j + 1], db[:],
                        remote_sem=rsem, local_sem=lsem, rdests=rd)
                trig = nc.gpsimd.trigger_dma(count=None)
                add_dep_helper(trig.ins, db_cp.ins, sync=True,
                               reason="fire peer writes only once d final")
                tb = mpool.tile([P, 1], F32, tag="tb")
                red = nc.vector.tensor_reduce(tb[:], ag[:], axis=AX.X,
                                              op=ALU.add)
                # pin the reduce after DVE's local ops so the post-
                # scheduling recv wait spliced before it cannot park DVE
                # mid-stream (that would deadlock the exchange globally)
                add_dep_helper(red.ins, db_cp.ins, sync=True,
                               reason="reduce after local d path")
                # registered directly so compile() still emits the prelude
                # AllGather that synchronizes the 8 kernel launches
                assert nc._bir_kernel_barrier_sem is not None
                nc._bir_kernel_barrier_sem_replica_groups.append(
                    set(range(NCORES)))
            else:
                nc.sync.dma_start(cc_in.ap(), d_loc[:])
                nc.gpsimd.collective_compute(
                    "AllGather", ALU.bypass,
                    replica_groups=[list(range(NCORES))],
                    ins=[cc_in.ap()], outs=[cc_out.ap()],
                )
                gath = mpool.tile([1, NCORES], F32, tag="gath")
                nc.sync.dma_start(gath[:], cc_out.ap())
                t_sc = mpool.tile([1, 1], F32, tag="s11c")
                nc.vector.tensor_reduce(t_sc[:], gath[:], axis=AX.X,
                                        op=ALU.add)
                ps_b = pspool.tile([P, 1], F32, tag="ps_small")
                nc.tensor.matmul(ps_b[:], ones_row[:], t_sc[:])
                tb = mpool.tile([P, 1], F32, tag="tb")
                nc.scalar.activation(tb[:], ps_b[:], ACT_FN.Copy)

            # ---------- fill ----------
            fill = cpool.tile([P, FILL_W], F32)
            nc.vector.tensor_scalar_mul(fill[:], ones_fill[:], tb[:])
            half = N_FILL // 2
            ovh = out.ap().rearrange("(h j p f) -> h p j f",
                                     h=2, p=P, f=FILL_W)
            fill_b = fill[:].unsqueeze(1).broadcast_to([P, half, FILL_W])
            # scalar's half first: its ring picks up ~2.5us later
            nc.scalar.dma_start(ovh[1], fill_b)
            nc.sync.dma_start(ovh[0], fill_b)

    if use_remote_exchange:
        # Recv wait, spliced in POST-scheduling directly before the reduce
        # (the scheduler deadlocks on a visible cross-core wait and hoists
        # a depless placeholder to the engine's idle front). Each inbound
        # peer write bumps rsem by 16 // n_dests = 2, so 7 peers => 14.
        w = nc.vector.wait_ge(rsem, 2 * (NCORES - 1))
        wins = w.ins
        fn = nc.m.functions[0]
        for blk in fn.blocks:
            il = blk.instructions
            for i in range(len(il)):
                if il[i] is wins:
                    del il[i]
                    break
        placed = False
        for blk in fn.blocks:
            il = blk.instructions
            for i in range(len(il)):
                if il[i] is red.ins:
                    il.insert(i, wins)
                    placed = True
                    break
            if placed:
                break
        assert placed, "could not splice recv wait before reduce"

    nc.compile()
    return nc


_NC_CACHE = None


def _get_nc():
    global _NC_CACHE
    if _NC_CACHE is None:
        _NC_CACHE = build_nc()
    return _NC_CACHE


def run(X_probs, Y_probs, H_bandwidth, trace=False, trace_kwargs=None):
    X = np.asarray(X_probs, dtype=np.float32).reshape(NCORES, ROWS_PER_CORE, FDIM)
    Y = np.asarray(Y_probs, dtype=np.float32).reshape(NCORES, ROWS_PER_CORE, HDIM)
    H = np.ascontiguousarray(np.asarray(H_bandwidth, dtype=np.float32))

    C = np.empty((NCORES, ROWS_PER_CORE, Q), dtype=np.float32)
    C[:, :, :FDIM] = X
    C[:, :, FDIM:] = Y

    nc = _get_nc()
    in_maps = [{"c": C[i], "h": H} for i in range(NCORES)]
    res = run_bass_kernel_spmd(nc, in_maps, list(range(NCORES)),
                               trace=trace, **(trace_kwargs or {}))
    full = np.concatenate([res.results[i]["o"] for i in range(NCORES)])
    return full.reshape((KGRID,) * HOUT), res


def kernel(X_probs, Y_probs, H_bandwidth, K, H_out):
    assert int(K) == KGRID and int(H_out) == HOUT
    out, _ = run(X_probs, Y_probs, H_bandwidth, trace=False)
    return out


# revision 18
# speedup vs baseline: 1.0883x; 1.0883x over previous
"""DiscreteKDE kernel for 8 Trainium2 NeuronCores.

Full computation:
    Q = 64; H_I = inv(H_bandwidth)
    Z  = (idx[:,None]-idx[None,:]) @ H_I
    KW = (1/sqrt(2pi)) * exp(-0.5 * Z*Z)
    col_sums = concat([X_probs.sum(0), Y_probs.sum(0)])     # (64,)  <- 256MB read
    T  = dot(KW.sum(0), col_sums)
    out = T * jnp.ones((256,256,256))                        # 67MB write

Hard-won structure notes (per-core):
  - HBM read throughput is ~335-358 GB/s per core NO MATTER HOW MANY
    HWDGE rings issue (measured: one ring 335; two rings on disjoint
    tiles 340; two rings split within a tile 220 -- the same-partition
    descriptor pairs collide). So the 32MB stream rides ONE ring (sync)
    as 15 x [128, 4096] tiles (16KB descs, fan-out 16), strict FIFO so
    DVE's in-order consume never stalls on a lagging second ring.
  - The 0.5MB tails ([128,1024] + [72,64]) ride the otherwise-idle
    scalar ring EARLY, so the sync FIFO ends with tile 14 and nothing
    small trails the stream.
  - Small inputs (64x64) go on the gpsimd SWDGE ring: 256B descriptors
    would eat ~2.9us of HWDGE issue each and delay the stream start.
  - DVE fp32 tensor_tensor is ~1.05ns/elem/lane + ~154ns fixed, so
    tiles 0-13 are consumed as [128,2048] wide adds into acc_w
    (2.3us/MB = 455GB/s > stream rate; v1's [128,512] adds made DVE a
    co-bottleneck). Tiles 13+14 are col-split (2x8KB / 4x4KB sub-DMAs)
    and consumed narrow into acc2 [128,512] so the narrow adds chase
    the stream per-slice and the LAST PSUM fold is a single matmul:
    the wide fold (4 matmuls, ones_k^T @ acc_w chunks accumulating
    into ps_pe [1,512]) hides behind tile 12, the acc2 fold closes
    the accumulation.
  - The dot d = <rep8(KW.sum(0)), ps_pe> runs on DVE reading PSUM
    directly (mul + X-reduce); the 128-partition broadcast of d comes
    back via PE and lands in SBUF via a DVE copy (gpsimd can't touch
    PSUM but must source the remote writes from SBUF).
  - Newton-Schulz inverse of H on PE+ACT (iterating on the negated
    inverse R' = R A R + 2R), alpha chain on gpsimd/ACT (1/x as
    exp(-ln x)); all done by ~35us, far off the critical path.
  - cross-core sum of the per-core dot d_r: HAND-ROLLED flat all-gather
    via 7 remote_dma_broadcast preps (dest j in slot j so cross-die
    dests ride D2D lanes), one trigger_dma gated on d (add_dep_helper;
    remote preps are "user-synced" so the framework defers NOTHING),
    then a DVE reduce of the [128, 8] gather tile. The recv wait
    (rsem >= 14; each inbound write bumps +2) is spliced into the
    instruction list POST-scheduling: the single-core tile scheduler
    deadlocks on a visible cross-core wait and hoists a depless
    placeholder to the engine's idle front (both tried, both failed).
    Replaces the runtime AllGather whose small-payload latency floor is
    ~20us (trigger delay + mesh walk) with ~2-4us of peer SBUF writes.
  - fill: [128, 4096] tile * T (16KB descs), one 4MB broadcast-AP DMA
    per ring; HBM WRITES do reach ~420GB/s across two rings.
  - Launch skew between the 8 PJRT dispatches is ~5-6us/core (varies);
    every core waits for the straggler's d, so the first-launched core
    pays the full spread. Unfixable from inside one SPMD program;
    everything else is tuned so the straggler's own timeline is short.
"""

import os
import sys

import numpy as np

for _p in ("/opt/trn_rl_repo", "/root/.axon_site/_ro/trn_rl_repo"):
    if os.path.isdir(_p) and _p not in sys.path:
        sys.path.insert(0, _p)

import concourse.bacc as bacc
import concourse.bass as bass
import concourse.mybir as mybir
from concourse.bass_utils import run_bass_kernel_spmd
from concourse.tile import TileContext
from concourse.tile_rust import add_dep_helper

# ---- problem constants (hardcoded per spec) ----
N_TOTAL = 1_000_000
FDIM = 61
HDIM = 3
Q = 64                      # FDIM + HDIM
KGRID = 256
HOUT = 3
NCORES = 8
ROWS_PER_CORE = N_TOTAL // NCORES          # 125000

# ---- tiling ----
P = 128
G = 64                      # rows/partition/tile -> 16KB descriptors
NT = 15                     # full tiles: 15 * 128 * 64 = 122880 rows
TW = G * Q                  # 4096 f32 = 16KB per partition per tile
WCH = 2048                  # wide DVE chunk
CH = 512                    # narrow chunk = residual-group space
NCHUNK = TW // CH           # 8
MAIN_ROWS = NT * P * G      # 122880
TAILA_G = 16                # [128, 1024]: 2048 rows, 4KB descriptors
TAILA_ROWS = P * TAILA_G    # 2048
TAILB_ROWS = ROWS_PER_CORE - MAIN_ROWS - TAILA_ROWS   # 72
STREAM_BUFS = 7
LAST_TILE = NT - 1          # consumed narrow into acc2

OUT_TOTAL = KGRID ** HOUT                  # 16_777_216
OUT_PER_CORE = OUT_TOTAL // NCORES         # 2_097_152
# 2048 (8KB descs) not 4096: same chip-capped fill bandwidth (issue rate
# is not the limiter), but the post-wait T-multiply halves to ~1.2us
FILL_W = 2048
N_FILL = OUT_PER_CORE // (P * FILL_W)      # 8

NEWTON_ITERS = 11
INV_SQRT_2PI = 0.3989422804014327
LN_C = float(np.log(INV_SQRT_2PI))

F32 = mybir.dt.float32
AX = mybir.AxisListType
ALU = mybir.AluOpType
ACT_FN = mybir.ActivationFunctionType


def build_nc(use_remote_exchange=True):
    nc = bacc.Bacc("TRN2", target_bir_lowering=False, debug=False,
                   num_devices=NCORES)

    c_in = nc.dram_tensor("c", [ROWS_PER_CORE, Q], F32, kind="ExternalInput")
    h_in = nc.dram_tensor("h", [Q, Q], F32, kind="ExternalInput")
    out = nc.dram_tensor("o", [OUT_PER_CORE], F32, kind="ExternalOutput")

    idx = np.arange(Q, dtype=np.float64)
    d_const = nc.inline_tensor(
        (idx[:, None] - idx[None, :]).astype(np.float32), "dmat")
    i2_const = nc.inline_tensor(
        (2.0 * np.eye(Q)).astype(np.float32), "i2mat")
    n2_const = nc.inline_tensor(
        (-2.0 * np.eye(Q)).astype(np.float32), "n2mat")

    if not use_remote_exchange:
        cc_in = nc.dram_tensor("cc_in", [1], F32)
        cc_out = nc.dram_tensor("cc_out", [NCORES], F32, addr_space="Shared")

    with TileContext(nc) as tc:
        with (
            tc.tile_pool(name="const", bufs=1) as cpool,
            tc.tile_pool(name="stream", bufs=STREAM_BUFS) as spool,
            tc.tile_pool(name="small", bufs=2) as mpool,
            tc.tile_pool(name="accp", bufs=1, space=bass.MemorySpace.PSUM) as ppool,
            tc.tile_pool(name="psmall", bufs=2, space=bass.MemorySpace.PSUM) as pspool,
        ):
            # ---------- stream DMAs (sync ring, strict FIFO) ----------
            cv = c_in.ap()[:MAIN_ROWS, :].rearrange(
                "(t p g) q -> t p (g q)", t=NT, p=P, g=G)
            taila_v = c_in.ap()[MAIN_ROWS:MAIN_ROWS + TAILA_ROWS, :].rearrange(
                "(p g) q -> p (g q)", p=P, g=TAILA_G)
            tailb_v = c_in.ap()[MAIN_ROWS + TAILA_ROWS:, :]

            tiles = []
            taila_t = cpool.tile([P, TAILA_G * Q], F32)
            tailb_t = cpool.tile([TAILB_ROWS, Q], F32)
            for t in range(NT):
                st = spool.tile([P, TW], F32, tag="stream")
                if t == LAST_TILE:
                    # column-split the last tile into sub-DMAs (4KB/2KB
                    # descs; issue-ahead hides the desc-rate cost) so the
                    # narrow DVE adds chase the stream; the final two
                    # slices are single 512-col chunks so the last
                    # dependency is ONE narrow add, not two
                    for lo, w in ((0, 1024), (1024, 1024), (2048, 1024),
                                  (3072, 512), (3584, 512)):
                        nc.sync.dma_start(st[:, lo:lo + w],
                                          cv[t][:, lo:lo + w])
                elif t == LAST_TILE - 1:
                    # t13 col-split in halves and consumed narrow too, so
                    # the 4-matmul acc_w fold hides behind t12 instead of
                    # gating the tail
                    for b in range(2):
                        nc.sync.dma_start(st[:, b * 2048:(b + 1) * 2048],
                                          cv[t][:, b * 2048:(b + 1) * 2048])
                else:
                    nc.sync.dma_start(st[:], cv[t])
                tiles.append(st)
                if t == 0:
                    # tails on the idle scalar ring, arriving early;
                    # dedicated buffers, consumed on arrival
                    nc.scalar.dma_start(taila_t[:], taila_v)
                    nc.scalar.dma_start(tailb_t[:], tailb_v)

            # ---------- constants (gpsimd: DVE streams, ACT does Newton) --
            ones_k = cpool.tile([P, 1], F32)        # lhsT partition-reduce
            nc.gpsimd.memset(ones_k[:], 1.0)
            ones_row = cpool.tile([1, P], F32)      # lhsT bcast scalar->128
            nc.gpsimd.memset(ones_row[:], 1.0)
            ones_q = cpool.tile([Q, 1], F32)        # lhsT 64-part reduce
            nc.gpsimd.memset(ones_q[:], 1.0)
            ones_rq = cpool.tile([1, Q], F32)       # lhsT bcast scalar->64
            nc.gpsimd.memset(ones_rq[:], 1.0)
            lnc = cpool.tile([Q, 1], F32)           # exp bias ln(1/sqrt 2pi)
            nc.gpsimd.memset(lnc[:], LN_C)
            ones_fill = cpool.tile([P, FILL_W], F32)
            nc.gpsimd.memset(ones_fill[:], 1.0)

            # ---------- small inputs (gpsimd SWDGE: tiny descriptors) -----
            a_t = cpool.tile([Q, Q], F32)
            nc.gpsimd.dma_start(a_t[:], h_in.ap())
            d_t = cpool.tile([Q, Q], F32)
            nc.gpsimd.dma_start(d_t[:], d_const.ap())
            i2_t = cpool.tile([Q, Q], F32)
            nc.gpsimd.dma_start(i2_t[:], i2_const.ap())
            n2_t = cpool.tile([Q, Q], F32)
            nc.gpsimd.dma_start(n2_t[:], n2_const.ap())

            # ---------- exchange state ----------
            if use_remote_exchange:
                rsem = nc.alloc_semaphore("xg_rsem")
                lsem = nc.alloc_semaphore("xg_lsem")
                ag = cpool.tile([P, NCORES], F32)    # gathered d_r
                db = cpool.tile([P, 1], F32)         # my d on 128 parts

            # ---------- alpha chain (gpsimd + ACT + PE) ----------
            # tmp_qq = A * 2I is diagonal => 2*trace = full-tensor sum,
            # exactly gpsimd's XYZWC (partition-inclusive) reduce.
            tmp_qq = mpool.tile([Q, Q], F32, tag="qq")
            nc.gpsimd.tensor_mul(tmp_qq[:], a_t[:], i2_t[:])
            tr_s = mpool.tile([1, 1], F32, tag="q1")
            nc.gpsimd.tensor_reduce(tr_s[:], tmp_qq[:], axis=AX.XYZWC,
                                    op=ALU.add)
            ln_t = mpool.tile([1, 1], F32, tag="s11ln")
            nc.scalar.activation(ln_t[:], tr_s[:], ACT_FN.Ln)
            tr2 = mpool.tile([1, 1], F32, tag="s11")
            nc.scalar.activation(tr2[:], ln_t[:], ACT_FN.Exp, scale=-1.0)
            ps_a = pspool.tile([Q, 1], F32, tag="ps_small")
            nc.tensor.matmul(ps_a[:], ones_rq[:], tr2[:])     # bcast->(64,1)
            al64 = mpool.tile([Q, 1], F32, tag="q1b")
            nc.scalar.activation(al64[:], ps_a[:], ACT_FN.Copy)

            # ---------- Newton-Schulz on PE + ACT ----------
            s_cur = mpool.tile([Q, Q], F32, tag="newton")
            nc.gpsimd.tensor_scalar_mul(s_cur[:], n2_t[:], al64[:])
            for it in range(NEWTON_ITERS):
                ps_y = pspool.tile([Q, Q], F32, tag="ps_qq")
                nc.tensor.matmul(ps_y[:], a_t[:], s_cur[:])       # A @ R
                y_sb = mpool.tile([Q, Q], F32, tag="newton_y")
                nc.scalar.activation(y_sb[:], ps_y[:], ACT_FN.Copy)
                ps_x = pspool.tile([Q, Q], F32, tag="ps_qq")
                nc.tensor.matmul(ps_x[:], s_cur[:], y_sb[:],
                                 start=True, stop=False)          # R A R
                nc.tensor.matmul(ps_x[:], s_cur[:], i2_t[:],
                                 start=False, stop=True)          # + 2 R
                s_nxt = mpool.tile([Q, Q], F32, tag="newton")
                nc.scalar.activation(s_nxt[:], ps_x[:], ACT_FN.Copy)
                s_cur = s_nxt
            # Z = D.T @ (-H^-1) up to sign; KW = exp(-Z^2/2 + ln c)
            ps_z = pspool.tile([Q, Q], F32, tag="ps_qq")
            nc.tensor.matmul(ps_z[:], d_t[:], s_cur[:])
            z2 = mpool.tile([Q, Q], F32, tag="qq2")
            nc.scalar.square(z2[:], ps_z[:])
            kw = mpool.tile([Q, Q], F32, tag="qq3")
            nc.scalar.activation(kw[:], z2[:], ACT_FN.Exp,
                                 bias=lnc[:], scale=-0.5)
            ps_s = pspool.tile([1, Q], F32, tag="ps_small")
            nc.tensor.matmul(ps_s[:], ones_q[:], kw[:])           # KW.sum(0)
            # replicate KW.sum(0) 8x into the 512-wide residual space
            s_rep = mpool.tile([1, NCHUNK * Q], F32, tag="vec2")
            s_rep_v = s_rep[:].rearrange("p (g q) -> p g q", g=NCHUNK, q=Q)
            ps_s_b = ps_s[:].unsqueeze(1).broadcast_to([1, NCHUNK, Q])
            nc.scalar.activation(s_rep_v, ps_s_b, ACT_FN.Copy)

            # ---------- DVE consume: wide adds, narrow tail ----------
            acc_w = cpool.tile([P, WCH], F32)       # wide acc (t0..t13)
            acc2 = cpool.tile([P, CH], F32)         # narrow acc (tails+t14)
            ps_pe = ppool.tile([1, CH], F32)        # final accumulation

            last_dve = [None]
            dve_w = [0]
            acc2_n = [0]

            def dve_wide(tile):
                for b in range(TW // WCH):
                    sl = tile[:, b * WCH:(b + 1) * WCH]
                    if dve_w[0] == 0:
                        last_dve[0] = nc.vector.tensor_copy(acc_w[:], sl)
                    else:
                        last_dve[0] = nc.vector.tensor_add(
                            acc_w[:], acc_w[:], sl)
                    dve_w[0] += 1

            def dve_narrow(tile, ncols):
                for b in range(ncols // CH):
                    sl = tile[:, b * CH:(b + 1) * CH]
                    if acc2_n[0] == 0:
                        last_dve[0] = nc.vector.tensor_copy(acc2[:], sl)
                    else:
                        last_dve[0] = nc.vector.tensor_add(
                            acc2[:], acc2[:], sl)
                    acc2_n[0] += 1

            for t in range(NT - 2):
                dve_wide(tiles[t])
                if t == 1:
                    # tails arrived early on the scalar ring
                    dve_narrow(taila_t, TAILA_G * Q)
                    # tailB: 72 rows of 64 cols, q-aligned on acc2 group 0
                    last_dve[0] = nc.vector.tensor_add(
                        acc2[:TAILB_ROWS, :Q], acc2[:TAILB_ROWS, :Q],
                        tailb_t[:])
                    acc2_n[0] += 1
            # wide-acc fold: 4 matmuls, hidden behind the t13/t14 stream
            for m in range(WCH // CH):
                nc.tensor.matmul(ps_pe[:], ones_k[:],
                                 acc_w[:, m * CH:(m + 1) * CH],
                                 start=(m == 0), stop=False)
            # last two tiles narrow so the final fold is one matmul
            dve_narrow(tiles[LAST_TILE - 1], TW)
            dve_narrow(tiles[LAST_TILE], TW)
            nc.tensor.matmul(ps_pe[:], ones_k[:], acc2[:],
                             start=False, stop=True)

            # ---------- local dot d = <KW.sum(0), col sums> (DVE) ---------
            # (tensor_tensor_reduce would fuse these two ops but dies on
            # hardware -- JaxRuntimeError INTERNAL -- keep mul + reduce)
            dprod = mpool.tile([1, NCHUNK * Q], F32, tag="vec2b")
            nc.vector.tensor_mul(dprod[:], s_rep[:], ps_pe[:])
            d_loc = mpool.tile([1, 1], F32, tag="s11d")
            dred = nc.vector.tensor_reduce(d_loc[:], dprod[:], axis=AX.X,
                                           op=ALU.add)
            last_dve[0] = dred

            if use_remote_exchange:
                # broadcast d to 128 partitions, then fire the peer writes
                ps_b = pspool.tile([P, 1], F32, tag="ps_small")
                nc.tensor.matmul(ps_b[:], ones_row[:], d_loc[:])
                db_cp = nc.vector.tensor_copy(db[:], ps_b[:])
                nc.gpsimd.tensor_copy(ag[:, 0:1], db[:])          # self slot
                # preps are descriptor-gen only; the SDMA reads db when the
                # trigger fires, so only the trigger needs the data dep.
                for j in range(1, NCORES):
                    rd = [None] * NCORES
                    rd[j] = (0, j)
                    nc.gpsimd.remote_dma_broadcast(
                        ag[:, j:j + 1], db[:],
                        remote_sem=rsem, local_sem=lsem, rdests=rd)
                trig = nc.gpsimd.trigger_dma(count=None)
                add_dep_helper(trig.ins, db_cp.ins, sync=True,
                               reason="fire peer writes only once d final")
                tb = mpool.tile([P, 1], F32, tag="tb")
                red = nc.vector.tensor_reduce(tb[:], ag[:], axis=AX.X,
                                              op=ALU.add)
                # pin the reduce after DVE's local ops so the post-
                # scheduling recv wait spliced before it cannot park DVE
                # mid-stream (that would deadlock the exchange globally)
                add_dep_helper(red.ins, db_cp.ins, sync=True,
                               reason="reduce after local d path")
                # registered directly so compile() still emits the prelude
                # AllGather that synchronizes the 8 kernel launches
                assert nc._bir_kernel_barrier_sem is not None
                nc._bir_kernel_barrier_sem_replica_groups.append(
                    set(range(NCORES)))
            else:
                nc.sync.dma_start(cc_in.ap(), d_loc[:])
                nc.gpsimd.collective_compute(
                    "AllGather", ALU.bypass,
                    replica_groups=[list(range(NCORES))],
                    ins=[cc_in.ap()], outs=[cc_out.ap()],
                )
                gath = mpool.tile([1, NCORES], F32, tag="gath")
                nc.sync.dma_start(gath[:], cc_out.ap())
                t_sc = mpool.tile([1, 1], F32, tag="s11c")
                nc.vector.tensor_reduce(t_sc[:], gath[:], axis=AX.X,
                                        op=ALU.add)
                ps_b = pspool.tile([P, 1], F32, tag="ps_small")
                nc.tensor.matmul(ps_b[:], ones_row[:], t_sc[:])
                tb = mpool.tile([P, 1], F32, tag="tb")
                nc.scalar.activation(tb[:], ps_b[:], ACT_FN.Copy)

            # ---------- fill ----------
            fill = cpool.tile([P, FILL_W], F32)
            nc.vector.tensor_scalar_mul(fill[:], ones_fill[:], tb[:])
            half = N_FILL // 2
            ovh = out.ap().rearrange("(h j p f) -> h p j f",
                                     h=2, p=P, f=FILL_W)
            fill_b = fill[:].unsqueeze(1).broadcast_to([P, half, FILL_W])
            # scalar's half first: its ring picks up ~2.5us later
            nc.scalar.dma_start(ovh[1], fill_b)
            nc.sync.dma_start(ovh[0], fill_b)

    if use_remote_exchange:
        # Recv wait, spliced in POST-scheduling directly before the reduce
        # (the scheduler deadlocks on a visible cross-core wait and hoists
        # a depless placeholder to the engine's idle front). Each inbound
        # peer write bumps rsem by 16 // n_dests = 2, so 7 peers => 14.
        w = nc.vector.wait_ge(rsem, 2 * (NCORES - 1))
        wins = w.ins
        fn = nc.m.functions[0]
        for blk in fn.blocks:
            il = blk.instructions
            for i in range(len(il)):
                if il[i] is wins:
                    del il[i]
                    break
        placed = False
        for blk in fn.blocks:
            il = blk.instructions
            for i in range(len(il)):
                if il[i] is red.ins:
                    il.insert(i, wins)
                    placed = True
                    break
            if placed:
                break
        assert placed, "could not splice recv wait before reduce"

    nc.compile()
    return nc


_NC_CACHE = None


def _get_nc():
    global _NC_CACHE
    if _NC_CACHE is None:
        _NC_CACHE = build_nc()
    return _NC_CACHE


def run(X_probs, Y_probs, H_bandwidth, trace=False, trace_kwargs=None):
    X = np.asarray(X_probs, dtype=np.float32).reshape(NCORES, ROWS_PER_CORE, FDIM)
    Y = np.asarray(Y_probs, dtype=np.float32).reshape(NCORES, ROWS_PER_CORE, HDIM)
    H = np.ascontiguousarray(np.asarray(H_bandwidth, dtype=np.float32))

    C = np.empty((NCORES, ROWS_PER_CORE, Q), dtype=np.float32)
    C[:, :, :FDIM] = X
    C[:, :, FDIM:] = Y

    nc = _get_nc()
    in_maps = [{"c": C[i], "h": H} for i in range(NCORES)]
    res = run_bass_kernel_spmd(nc, in_maps, list(range(NCORES)),
                               trace=trace, **(trace_kwargs or {}))
    full = np.concatenate([res.results[i]["o"] for i in range(NCORES)])
    return full.reshape((KGRID,) * HOUT), res


def kernel(X_probs, Y_probs, H_bandwidth, K, H_out):
    assert int(K) == KGRID and int(H_out) == HOUT
    out, _ = run(X_probs, Y_probs, H_bandwidth, trace=False)
    return out


# revision 19
# speedup vs baseline: 1.1593x; 1.0652x over previous
"""DiscreteKDE kernel for 8 Trainium2 NeuronCores.

Full computation:
    Q = 64; H_I = inv(H_bandwidth)
    Z  = (idx[:,None]-idx[None,:]) @ H_I
    KW = (1/sqrt(2pi)) * exp(-0.5 * Z*Z)
    col_sums = concat([X_probs.sum(0), Y_probs.sum(0)])     # (64,)  <- 256MB read
    T  = dot(KW.sum(0), col_sums)
    out = T * jnp.ones((256,256,256))                        # 67MB write

Hard-won structure notes (per-core):
  - HBM read throughput is ~335-358 GB/s per core NO MATTER HOW MANY
    HWDGE rings issue (measured: one ring 335; two rings on disjoint
    tiles 340; two rings split within a tile 220 -- the same-partition
    descriptor pairs collide). So the 32MB stream rides ONE ring (sync)
    as 15 x [128, 4096] tiles (16KB descs, fan-out 16), strict FIFO so
    DVE's in-order consume never stalls on a lagging second ring.
  - The 0.5MB tails ([128,1024] + [72,64]) ride the otherwise-idle
    scalar ring EARLY, so the sync FIFO ends with tile 14 and nothing
    small trails the stream.
  - Small inputs (64x64) go on the gpsimd SWDGE ring: 256B descriptors
    would eat ~2.9us of HWDGE issue each and delay the stream start.
  - DVE fp32 tensor_tensor is ~1.05ns/elem/lane + ~154ns fixed, so
    tiles 0-13 are consumed as [128,2048] wide adds into acc_w
    (2.3us/MB = 455GB/s > stream rate; v1's [128,512] adds made DVE a
    co-bottleneck). Tiles 13+14 are col-split (2x8KB / 4x4KB sub-DMAs)
    and consumed narrow into acc2 [128,512] so the narrow adds chase
    the stream per-slice and the LAST PSUM fold is a single matmul:
    the wide fold (4 matmuls, ones_k^T @ acc_w chunks accumulating
    into ps_pe [1,512]) hides behind tile 12, the acc2 fold closes
    the accumulation.
  - The dot d = <rep8(KW.sum(0)), ps_pe> runs on DVE reading PSUM
    directly (mul + X-reduce); the 128-partition broadcast of d comes
    back via PE and lands in SBUF via a DVE copy (gpsimd can't touch
    PSUM but must source the remote writes from SBUF).
  - Newton-Schulz inverse of H on PE+ACT (iterating on the negated
    inverse R' = R A R + 2R), alpha chain on gpsimd/ACT (1/x as
    exp(-ln x)); all done by ~35us, far off the critical path.
  - cross-core sum of the per-core dot d_r: HAND-ROLLED flat all-gather
    via 7 remote_dma_broadcast preps (dest j in slot j so cross-die
    dests ride D2D lanes), one trigger_dma gated on d (add_dep_helper;
    remote preps are "user-synced" so the framework defers NOTHING),
    then a DVE reduce of the [128, 8] gather tile. The recv wait
    (rsem >= 14; each inbound write bumps +2) is spliced into the
    instruction list POST-scheduling: the single-core tile scheduler
    deadlocks on a visible cross-core wait and hoists a depless
    placeholder to the engine's idle front (both tried, both failed).
    Replaces the runtime AllGather whose small-payload latency floor is
    ~20us (trigger delay + mesh walk) with ~2-4us of peer SBUF writes.
  - fill: [128, 4096] tile * T (16KB descs), one 4MB broadcast-AP DMA
    per ring; HBM WRITES do reach ~420GB/s across two rings.
  - Launch skew between the 8 PJRT dispatches is ~5-6us/core (varies);
    every core waits for the straggler's d, so the first-launched core
    pays the full spread. Unfixable from inside one SPMD program;
    everything else is tuned so the straggler's own timeline is short.
"""

import os
import sys

import numpy as np

for _p in ("/opt/trn_rl_repo", "/root/.axon_site/_ro/trn_rl_repo"):
    if os.path.isdir(_p) and _p not in sys.path:
        sys.path.insert(0, _p)

import concourse.bacc as bacc
import concourse.bass as bass
import concourse.mybir as mybir
from concourse.bass_utils import run_bass_kernel_spmd
from concourse.tile import TileContext
from concourse.tile_rust import add_dep_helper

# ---- problem constants (hardcoded per spec) ----
N_TOTAL = 1_000_000
FDIM = 61
HDIM = 3
Q = 64                      # FDIM + HDIM
KGRID = 256
HOUT = 3
NCORES = 8
ROWS_PER_CORE = N_TOTAL // NCORES          # 125000

# ---- tiling ----
P = 128
G = 64                      # rows/partition/tile -> 16KB descriptors
NT = 15                     # full tiles: 15 * 128 * 64 = 122880 rows
TW = G * Q                  # 4096 f32 = 16KB per partition per tile
WCH = 2048                  # wide DVE chunk
CH = 512                    # narrow chunk = residual-group space
NCHUNK = TW // CH           # 8
MAIN_ROWS = NT * P * G      # 122880
TAILA_G = 16                # [128, 1024]: 2048 rows, 4KB descriptors
TAILA_ROWS = P * TAILA_G    # 2048
TAILB_ROWS = ROWS_PER_CORE - MAIN_ROWS - TAILA_ROWS   # 72
STREAM_BUFS = 7
LAST_TILE = NT - 1          # consumed narrow into acc2

OUT_TOTAL = KGRID ** HOUT                  # 16_777_216
OUT_PER_CORE = OUT_TOTAL // NCORES         # 2_097_152
# 2048 (8KB descs) not 4096: same chip-capped fill bandwidth (issue rate
# is not the limiter), but the post-wait T-multiply halves to ~1.2us
FILL_W = 2048
N_FILL = OUT_PER_CORE // (P * FILL_W)      # 8

NEWTON_ITERS = 11
INV_SQRT_2PI = 0.3989422804014327
LN_C = float(np.log(INV_SQRT_2PI))

F32 = mybir.dt.float32
AX = mybir.AxisListType
ALU = mybir.AluOpType
ACT_FN = mybir.ActivationFunctionType


def build_nc(use_remote_exchange=True):
    nc = bacc.Bacc("TRN2", target_bir_lowering=False, debug=False,
                   num_devices=NCORES)

    c_in = nc.dram_tensor("c", [ROWS_PER_CORE, Q], F32, kind="ExternalInput")
    h_in = nc.dram_tensor("h", [Q, Q], F32, kind="ExternalInput")
    out = nc.dram_tensor("o", [OUT_PER_CORE], F32, kind="ExternalOutput")

    idx = np.arange(Q, dtype=np.float64)
    d_const = nc.inline_tensor(
        (idx[:, None] - idx[None, :]).astype(np.float32), "dmat")
    i2_const = nc.inline_tensor(
        (2.0 * np.eye(Q)).astype(np.float32), "i2mat")
    n2_const = nc.inline_tensor(
        (-2.0 * np.eye(Q)).astype(np.float32), "n2mat")

    if not use_remote_exchange:
        cc_in = nc.dram_tensor("cc_in", [1], F32)
        cc_out = nc.dram_tensor("cc_out", [NCORES], F32, addr_space="Shared")

    with TileContext(nc) as tc:
        with (
            tc.tile_pool(name="const", bufs=1) as cpool,
            tc.tile_pool(name="stream", bufs=STREAM_BUFS) as spool,
            tc.tile_pool(name="small", bufs=2) as mpool,
            tc.tile_pool(name="accp", bufs=1, space=bass.MemorySpace.PSUM) as ppool,
            tc.tile_pool(name="psmall", bufs=2, space=bass.MemorySpace.PSUM) as pspool,
        ):
            # ---------- stream DMAs (sync ring, strict FIFO) ----------
            cv = c_in.ap()[:MAIN_ROWS, :].rearrange(
                "(t p g) q -> t p (g q)", t=NT, p=P, g=G)
            taila_v = c_in.ap()[MAIN_ROWS:MAIN_ROWS + TAILA_ROWS, :].rearrange(
                "(p g) q -> p (g q)", p=P, g=TAILA_G)
            tailb_v = c_in.ap()[MAIN_ROWS + TAILA_ROWS:, :]

            tiles = []
            taila_t = cpool.tile([P, TAILA_G * Q], F32)
            tailb_t = cpool.tile([TAILB_ROWS, Q], F32)
            for t in range(NT):
                st = spool.tile([P, TW], F32, tag="stream")
                if t == LAST_TILE:
                    # column-split the last tile into 4 sub-DMAs (4KB
                    # descs; issue-ahead hides the desc-rate cost) so the
                    # narrow DVE adds chase the stream instead of waiting
                    # for the whole 2MB to land
                    for b in range(4):
                        nc.sync.dma_start(st[:, b * 1024:(b + 1) * 1024],
                                          cv[t][:, b * 1024:(b + 1) * 1024])
                elif t == LAST_TILE - 1:
                    # t13 col-split in halves and consumed narrow too, so
                    # the 4-matmul acc_w fold hides behind t12 instead of
                    # gating the tail
                    for b in range(2):
                        nc.sync.dma_start(st[:, b * 2048:(b + 1) * 2048],
                                          cv[t][:, b * 2048:(b + 1) * 2048])
                else:
                    nc.sync.dma_start(st[:], cv[t])
                tiles.append(st)
                if t == 0:
                    # tails on the idle scalar ring, arriving early;
                    # dedicated buffers, consumed on arrival
                    nc.scalar.dma_start(taila_t[:], taila_v)
                    nc.scalar.dma_start(tailb_t[:], tailb_v)

            # ---------- constants (gpsimd: DVE streams, ACT does Newton) --
            ones_k = cpool.tile([P, 1], F32)        # lhsT partition-reduce
            nc.gpsimd.memset(ones_k[:], 1.0)
            ones_row = cpool.tile([1, P], F32)      # lhsT bcast scalar->128
            nc.gpsimd.memset(ones_row[:], 1.0)
            ones_q = cpool.tile([Q, 1], F32)        # lhsT 64-part reduce
            nc.gpsimd.memset(ones_q[:], 1.0)
            ones_rq = cpool.tile([1, Q], F32)       # lhsT bcast scalar->64
            nc.gpsimd.memset(ones_rq[:], 1.0)
            lnc = cpool.tile([Q, 1], F32)           # exp bias ln(1/sqrt 2pi)
            nc.gpsimd.memset(lnc[:], LN_C)
            ones_fill = cpool.tile([P, FILL_W], F32)
            nc.gpsimd.memset(ones_fill[:], 1.0)

            # ---------- small inputs (gpsimd SWDGE: tiny descriptors) -----
            a_t = cpool.tile([Q, Q], F32)
            nc.gpsimd.dma_start(a_t[:], h_in.ap())
            d_t = cpool.tile([Q, Q], F32)
            nc.gpsimd.dma_start(d_t[:], d_const.ap())
            i2_t = cpool.tile([Q, Q], F32)
            nc.gpsimd.dma_start(i2_t[:], i2_const.ap())
            n2_t = cpool.tile([Q, Q], F32)
            nc.gpsimd.dma_start(n2_t[:], n2_const.ap())

            # ---------- exchange state ----------
            if use_remote_exchange:
                rsem = nc.alloc_semaphore("xg_rsem")
                lsem = nc.alloc_semaphore("xg_lsem")
                ag = cpool.tile([P, NCORES], F32)    # gathered d_r
                db = cpool.tile([P, 1], F32)         # my d on 128 parts

            # ---------- alpha chain (gpsimd + ACT + PE) ----------
            # tmp_qq = A * 2I is diagonal => 2*trace = full-tensor sum,
            # exactly gpsimd's XYZWC (partition-inclusive) reduce.
            tmp_qq = mpool.tile([Q, Q], F32, tag="qq")
            nc.gpsimd.tensor_mul(tmp_qq[:], a_t[:], i2_t[:])
            tr_s = mpool.tile([1, 1], F32, tag="q1")
            nc.gpsimd.tensor_reduce(tr_s[:], tmp_qq[:], axis=AX.XYZWC,
                                    op=ALU.add)
            ln_t = mpool.tile([1, 1], F32, tag="s11ln")
            nc.scalar.activation(ln_t[:], tr_s[:], ACT_FN.Ln)
            tr2 = mpool.tile([1, 1], F32, tag="s11")
            nc.scalar.activation(tr2[:], ln_t[:], ACT_FN.Exp, scale=-1.0)
            ps_a = pspool.tile([Q, 1], F32, tag="ps_small")
            nc.tensor.matmul(ps_a[:], ones_rq[:], tr2[:])     # bcast->(64,1)
            al64 = mpool.tile([Q, 1], F32, tag="q1b")
            nc.scalar.activation(al64[:], ps_a[:], ACT_FN.Copy)

            # ---------- Newton-Schulz on PE + ACT ----------
            s_cur = mpool.tile([Q, Q], F32, tag="newton")
            nc.gpsimd.tensor_scalar_mul(s_cur[:], n2_t[:], al64[:])
            for it in range(NEWTON_ITERS):
                ps_y = pspool.tile([Q, Q], F32, tag="ps_qq")
                nc.tensor.matmul(ps_y[:], a_t[:], s_cur[:])       # A @ R
                y_sb = mpool.tile([Q, Q], F32, tag="newton_y")
                nc.scalar.activation(y_sb[:], ps_y[:], ACT_FN.Copy)
                ps_x = pspool.tile([Q, Q], F32, tag="ps_qq")
                nc.tensor.matmul(ps_x[:], s_cur[:], y_sb[:],
                                 start=True, stop=False)          # R A R
                nc.tensor.matmul(ps_x[:], s_cur[:], i2_t[:],
                                 start=False, stop=True)          # + 2 R
                s_nxt = mpool.tile([Q, Q], F32, tag="newton")
                nc.scalar.activation(s_nxt[:], ps_x[:], ACT_FN.Copy)
                s_cur = s_nxt
            # Z = D.T @ (-H^-1) up to sign; KW = exp(-Z^2/2 + ln c)
            ps_z = pspool.tile([Q, Q], F32, tag="ps_qq")
            nc.tensor.matmul(ps_z[:], d_t[:], s_cur[:])
            z2 = mpool.tile([Q, Q], F32, tag="qq2")
            nc.scalar.square(z2[:], ps_z[:])
            kw = mpool.tile([Q, Q], F32, tag="qq3")
            nc.scalar.activation(kw[:], z2[:], ACT_FN.Exp,
                                 bias=lnc[:], scale=-0.5)
            ps_s = pspool.tile([1, Q], F32, tag="ps_small")
            nc.tensor.matmul(ps_s[:], ones_q[:], kw[:])           # KW.sum(0)
            # replicate KW.sum(0) 8x into the 512-wide residual space
            s_rep = mpool.tile([1, NCHUNK * Q], F32, tag="vec2")
            s_rep_v = s_rep[:].rearrange("p (g q) -> p g q", g=NCHUNK, q=Q)
            ps_s_b = ps_s[:].unsqueeze(1).broadcast_to([1, NCHUNK, Q])
            nc.scalar.activation(s_rep_v, ps_s_b, ACT_FN.Copy)

            # ---------- DVE consume: wide adds, narrow tail ----------
            acc_w = cpool.tile([P, WCH], F32)       # wide acc (t0..t13)
            acc2 = cpool.tile([P, CH], F32)         # narrow acc (tails+t14)
            ps_pe = ppool.tile([1, CH], F32)        # final accumulation

            last_dve = [None]
            dve_w = [0]
            acc2_n = [0]

            def dve_wide(tile):
                for b in range(TW // WCH):
                    sl = tile[:, b * WCH:(b + 1) * WCH]
                    if dve_w[0] == 0:
                        last_dve[0] = nc.vector.tensor_copy(acc_w[:], sl)
                    else:
                        last_dve[0] = nc.vector.tensor_add(
                            acc_w[:], acc_w[:], sl)
                    dve_w[0] += 1

            def dve_narrow(tile, ncols):
                for b in range(ncols // CH):
                    sl = tile[:, b * CH:(b + 1) * CH]
                    if acc2_n[0] == 0:
                        last_dve[0] = nc.vector.tensor_copy(acc2[:], sl)
                    else:
                        last_dve[0] = nc.vector.tensor_add(
                            acc2[:], acc2[:], sl)
                    acc2_n[0] += 1

            for t in range(NT - 2):
                dve_wide(tiles[t])
                if t == 1:
                    # tails arrived early on the scalar ring
                    dve_narrow(taila_t, TAILA_G * Q)
                    # tailB: 72 rows of 64 cols, q-aligned on acc2 group 0
                    last_dve[0] = nc.vector.tensor_add(
                        acc2[:TAILB_ROWS, :Q], acc2[:TAILB_ROWS, :Q],
                        tailb_t[:])
                    acc2_n[0] += 1
            # wide-acc fold: 4 matmuls, hidden behind the t13/t14 stream
            for m in range(WCH // CH):
                nc.tensor.matmul(ps_pe[:], ones_k[:],
                                 acc_w[:, m * CH:(m + 1) * CH],
                                 start=(m == 0), stop=False)
            # last two tiles narrow so the final fold is one matmul
            dve_narrow(tiles[LAST_TILE - 1], TW)
            dve_narrow(tiles[LAST_TILE], TW)
            nc.tensor.matmul(ps_pe[:], ones_k[:], acc2[:],
                             start=False, stop=True)

            # ---------- local dot d = <KW.sum(0), col sums> (DVE) ---------
            # (tensor_tensor_reduce would fuse these two ops but dies on
            # hardware -- JaxRuntimeError INTERNAL -- keep mul + reduce)
            dprod = mpool.tile([1, NCHUNK * Q], F32, tag="vec2b")
            nc.vector.tensor_mul(dprod[:], s_rep[:], ps_pe[:])
            d_loc = mpool.tile([1, 1], F32, tag="s11d")
            dred = nc.vector.tensor_reduce(d_loc[:], dprod[:], axis=AX.X,
                                           op=ALU.add)
            last_dve[0] = dred

            if use_remote_exchange:
                # broadcast d to 128 partitions, then fire the peer writes
                ps_b = pspool.tile([P, 1], F32, tag="ps_small")
                nc.tensor.matmul(ps_b[:], ones_row[:], d_loc[:])
                db_cp = nc.vector.tensor_copy(db[:], ps_b[:])
                nc.gpsimd.tensor_copy(ag[:, 0:1], db[:])          # self slot
                # preps are descriptor-gen only; the SDMA reads db when the
                # trigger fires, so only the trigger needs the data dep.
                for j in range(1, NCORES):
                    rd = [None] * NCORES
                    rd[j] = (0, j)
                    nc.gpsimd.remote_dma_broadcast(
                        ag[:, j:j + 1], db[:],
                        remote_sem=rsem, local_sem=lsem, rdests=rd)
                trig = nc.gpsimd.trigger_dma(count=None)
                add_dep_helper(trig.ins, db_cp.ins, sync=True,
                               reason="fire peer writes only once d final")
                tb = mpool.tile([P, 1], F32, tag="tb")
                red = nc.vector.tensor_reduce(tb[:], ag[:], axis=AX.X,
                                              op=ALU.add)
                # pin the reduce after DVE's local ops so the post-
                # scheduling recv wait spliced before it cannot park DVE
                # mid-stream (that would deadlock the exchange globally)
                add_dep_helper(red.ins, db_cp.ins, sync=True,
                               reason="reduce after local d path")
                # registered directly so compile() still emits the prelude
                # AllGather that synchronizes the 8 kernel launches
                assert nc._bir_kernel_barrier_sem is not None
                nc._bir_kernel_barrier_sem_replica_groups.append(
                    set(range(NCORES)))
            else:
                nc.sync.dma_start(cc_in.ap(), d_loc[:])
                nc.gpsimd.collective_compute(
                    "AllGather", ALU.bypass,
                    replica_groups=[list(range(NCORES))],
                    ins=[cc_in.ap()], outs=[cc_out.ap()],
                )
                gath = mpool.tile([1, NCORES], F32, tag="gath")
                nc.sync.dma_start(gath[:], cc_out.ap())
                t_sc = mpool.tile([1, 1], F32, tag="s11c")
                nc.vector.tensor_reduce(t_sc[:], gath[:], axis=AX.X,
                                        op=ALU.add)
                ps_b = pspool.tile([P, 1], F32, tag="ps_small")
                nc.tensor.matmul(ps_b[:], ones_row[:], t_sc[:])
                tb = mpool.tile([P, 1], F32, tag="tb")
                nc.scalar.activation(tb[:], ps_b[:], ACT_FN.Copy)

            # ---------- fill ----------
            fill = cpool.tile([P, FILL_W], F32)
            nc.vector.tensor_scalar_mul(fill[:], ones_fill[:], tb[:])
            half = N_FILL // 2
            ovh = out.ap().rearrange("(h j p f) -> h p j f",
                                     h=2, p=P, f=FILL_W)
            fill_b = fill[:].unsqueeze(1).broadcast_to([P, half, FILL_W])
            # scalar's half first: its ring picks up ~2.5us later
            nc.scalar.dma_start(ovh[1], fill_b)
            nc.sync.dma_start(ovh[0], fill_b)

    if use_remote_exchange:
        # Recv wait, spliced in POST-scheduling directly before the reduce
        # (the scheduler deadlocks on a visible cross-core wait and hoists
        # a depless placeholder to the engine's idle front). Each inbound
        # peer write bumps rsem by 16 // n_dests = 2, so 7 peers => 14.
        w = nc.vector.wait_ge(rsem, 2 * (NCORES - 1))
        wins = w.ins
        fn = nc.m.functions[0]
        for blk in fn.blocks:
            il = blk.instructions
            for i in range(len(il)):
                if il[i] is wins:
                    del il[i]
                    break
        placed = False
        for blk in fn.blocks:
            il = blk.instructions
            for i in range(len(il)):
                if il[i] is red.ins:
                    il.insert(i, wins)
                    placed = True
                    break
            if placed:
                break
        assert placed, "could not splice recv wait before reduce"

    nc.compile()
    return nc


_NC_CACHE = None


def _get_nc():
    global _NC_CACHE
    if _NC_CACHE is None:
        _NC_CACHE = build_nc()
    return _NC_CACHE


def run(X_probs, Y_probs, H_bandwidth, trace=False, trace_kwargs=None):
    X = np.asarray(X_probs, dtype=np.float32).reshape(NCORES, ROWS_PER_CORE, FDIM)
    Y = np.asarray(Y_probs, dtype=np.float32).reshape(NCORES, ROWS_PER_CORE, HDIM)
    H = np.ascontiguousarray(np.asarray(H_bandwidth, dtype=np.float32))

    C = np.empty((NCORES, ROWS_PER_CORE, Q), dtype=np.float32)
    C[:, :, :FDIM] = X
    C[:, :, FDIM:] = Y

    nc = _get_nc()
    in_maps = [{"c": C[i], "h": H} for i in range(NCORES)]
    res = run_bass_kernel_spmd(nc, in_maps, list(range(NCORES)),
                               trace=trace, **(trace_kwargs or {}))
    full = np.concatenate([res.results[i]["o"] for i in range(NCORES)])
    return full.reshape((KGRID,) * HOUT), res


def kernel(X_probs, Y_probs, H_bandwidth, K, H_out):
    assert int(K) == KGRID and int(H_out) == HOUT
    out, _ = run(X_probs, Y_probs, H_bandwidth, trace=False)
    return out
